# revision 7
# baseline (speedup 1.0000x reference)
"""AudioAttNet Trainium2 kernel, v8.

Computation per batch element b (65536 total):
  x[29, 8] -> conv1d(29->16, k=3) + lrelu -> conv(16->8) + lrelu
           -> conv(8->4) + lrelu -> conv(4->128) + lrelu = y [128, 8]
  logits = y^T @ wl^T ; attn = softmax(logits, axis=seq)
  out = sum_seq(y^T * attn)  = [128]

Mapping: pure data parallel over batch across 8 cores (8192/core).
Host prep: x is converted to f16, transposed to [(c,s)=232(+ones row), B]
and padded with a constant-one row so conv1's bias rides the matmul.
All conv biases are folded into the matmuls (ones-rows); conv4 runs as
eight K=33 matmuls (32 taps + bias row from a persistent ones row in y3).

v8 (from v7 trace analysis, 256us baseline):
 - PE HAM heater: the PE's default clock state is throttled (K=4/8 =
   1.2 GHz); ~3.4us of back-to-back matmul activity un-throttles it to
   2.4 GHz.  v7 ran cold essentially the whole kernel.  A burst of
   dummy matmuls at kernel start (overlapping the initial weight/x DMAs
   and ACT table load, when the PE is idle anyway) warms the clock
   before conv1, and the restructured gap-free MM stream keeps it warm.
 - De-interleaved slack-1 pipeline: iteration i emits load(i+2) |
   conv123(i+1) | conv4(i) | linear(i-1) | tail(i-2); every stage
   consumes data produced >= 1 full iteration earlier, so no engine
   waits on a same-iteration producer.  Tail lag cut from 4 to 2
   (less pipeline fill/drain; v7 spent ~80us there).
 - Engine balance (steady state, per 1024-batch chunk): scalar does
   PSUM evacuations (fused bias+prelu / exp, ~1.15us per 1024-elem
   slice); vector does softmax tail + some offload; gpsimd (SBUF-only
   f16 adds at ~2ns/elem) takes tree-fold slices.  CC_SPLIT/CC_POLY/
   CC_C123 env knobs tune the assignment without code changes.

Note: gpsimd cannot access PSUM on TRN2, TensorScalarPtr is not legal on
gpsimd, and SWDGE accumulate-DMA faults the exec unit — all three were
tried and rejected against real hardware (v7).
"""

import os
import numpy as np
from contextlib import ExitStack

import concourse.bass as bass
from concourse import bacc
from concourse import mybir
from concourse.bass_utils import run_bass_kernel_spmd

F16 = mybir.dt.float16
F32 = mybir.dt.float32
AF = mybir.ActivationFunctionType
ALU = mybir.AluOpType

B, C, S = 65536, 29, 8
NCORES = 8
BPC = B // NCORES            # batches per core
BC = 1024                    # batches per chunk
NCHUNK = BPC // BC
CS = C * S                   # 232
XROWS = CS + 1               # +1 ones row for the conv1 bias
NEG = 0.02

# conv4 slices whose evac runs as a 2-op DVE sequence instead of scalar
_SPLIT = set(int(c) for c in os.environ.get("CC_SPLIT", ""))
# linear slices whose exp runs on the vector engine as a polynomial
_POLY = set(int(c) for c in os.environ.get("CC_POLY", "24"))
# engine for the c1/c2/c3 evacuations: "scalar" or "dve"
_C123 = os.environ.get("CC_C123", "scalar")
_HEAT = int(os.environ.get("CC_HEAT", "16"))


def _build_nc():
    nc = bacc.Bacc()

    x_in = nc.declare_dram_parameter("xt", [XROWS, BPC], F16, isOutput=False)
    w1a_d = nc.declare_dram_parameter("w1a", [128, 128], F16, isOutput=False)
    w1b_d = nc.declare_dram_parameter("w1b", [105, 128], F16, isOutput=False)
    w2_d = nc.declare_dram_parameter("w2e", [128, 64], F16, isOutput=False)
    w3_d = nc.declare_dram_parameter("w3e", [64, 32], F16, isOutput=False)
    w4_d = nc.declare_dram_parameter("w4s", [33, 8 * 128], F16, isOutput=False)
    wl_d = nc.declare_dram_parameter("wlt", [128, 128], F16, isOutput=False)
    b2_d = nc.declare_dram_parameter("b2v", [64, 1], F32, isOutput=False)
    b3_d = nc.declare_dram_parameter("b3v", [32, 1], F32, isOutput=False)
    out_d = nc.declare_dram_parameter("out", [128, BPC], F16, isOutput=True)

    from concourse.tile import TileContext

    with TileContext(nc) as tc, ExitStack() as ctx:
        consts = ctx.enter_context(tc.tile_pool(name="consts", bufs=1))
        w1a = consts.tile_from(w1a_d[:])
        w1b = consts.tile_from(w1b_d[:])
        w2e = consts.tile_from(w2_d[:])
        w3e = consts.tile_from(w3_d[:])
        w4s_flat = consts.tile_from(w4_d[:])
        w4s = w4s_flat[:].rearrange("p (s d) -> p s d", s=8)
        wlt = consts.tile_from(wl_d[:])
        b2v = consts.tile_from(b2_d[:])
        b3v = consts.tile_from(b3_d[:])
        alpha_v = consts.tile([128, 1], F32)
        nc.vector.memset(alpha_v[:], NEG)
        # warm the Exp/Prelu activation table before the first conv
        warm = consts.tile([1, 1], F16)
        nc.scalar.activation(warm[:], alpha_v[0:1, :], AF.Exp)

        # persistent, manually double-buffered tiles (ones rows set once)
        y3_bufs = [consts.tile([33, BC], F16, name=f"y3_{i}") for i in range(2)]
        for t in y3_bufs:
            nc.vector.memset(t[32:33, :], 1.0)

        io = ctx.enter_context(tc.tile_pool(name="io", bufs=3))
        acts = ctx.enter_context(tc.tile_pool(name="acts", bufs=2))
        big = ctx.enter_context(tc.tile_pool(name="bigsb", bufs=3))
        tail = ctx.enter_context(tc.tile_pool(name="tailp", bufs=2))
        # one unified PSUM ring: 2 x [128, 4, 512] f32 = all 8 banks; every
        # matmul group (c123 / conv4 s-pairs / linear s-pairs) rotates
        # through it, so conv4/linear evacuations run at FD=2048.
        psA = ctx.enter_context(tc.tile_pool(name="psA", bufs=2, space="PSUM"))

        # ---- PE heater: un-throttle the HAM clock gate before conv1 ----
        if _HEAT:
            hw = consts.tile([128, 128], F16, name="heatw")
            hr = consts.tile([128, 512], F16, name="heatr")
            nc.vector.memset(hw[:], 0.0)
            nc.vector.memset(hr[:], 0.0)
            ph = psA.tile([128, 4, 512], F32, tag="psA", name="pheat")
            for _ in range(_HEAT):
                nc.tensor.matmul(ph[:, 0], hw[:], hr[:], start=True, stop=True)

        def evac_prelu(eng, dst, src, nslice):
            """dst = lrelu(src) (bias already in src). src is an f32 psum
            AP; dst a matching f16 AP."""
            if eng == "scalar":
                nc.scalar.activation(dst, src, AF.Prelu,
                                     alpha=alpha_v[0:src.shape[0], :])
            else:  # dve: t = 0.02*psum; dst = max(t, psum)
                a = src.shape[1]
                tmp = tail.tile([128, 2 * BC], F16, tag="tmps",
                                name=f"tmp{nslice}", bufs=3)
                tv = tmp[:src.shape[0], 0:a * 512].rearrange(
                    "p (a b) -> p a b", a=a)
                nc.vector.tensor_scalar(tv, src, NEG, None, ALU.mult)
                nc.vector.tensor_max(dst, tv, src)

        def emit_exp(k, dst, pl):
            """dst = exp(pl). Slices in _POLY run on the vector engine as
            (0.5(l/2+1)^2+0.5)^2 (|l|<0.4 -> rel err < 2e-3), relieving the
            scalar engine; the rest use the Exp table.  dst is [p, a, 512]
            with a in {2, 4}; pl the matching psum view."""
            if k in _POLY:
                a = dst.shape[1]
                q = tail.tile([128, 2 * BC], F16, tag="tmps", name=f"q{k}",
                              bufs=3)
                qv = q[:, 0:a * 512].rearrange("p (a b) -> p a b", a=a)
                nc.vector.tensor_scalar(qv, pl, 0.5, 1.0, ALU.mult,
                                        ALU.add)
                nc.vector.tensor_mul(qv, qv, qv)
                nc.vector.tensor_scalar(qv, qv, 0.5, 0.5, ALU.mult, ALU.add)
                nc.vector.tensor_mul(dst, qv, qv)
            else:
                nc.scalar.activation(dst, pl, AF.Exp)

        def load(ch):
            xt1 = io.tile([128, BC], F16, tag="xt1", name="xt1")
            xt2 = io.tile([105, BC], F16, tag="xt2", name="xt2")
            sl = slice(ch * BC, (ch + 1) * BC)
            nc.sync.dma_start(out=xt1[:], in_=x_in[0:128, sl])
            nc.sync.dma_start(out=xt2[:], in_=x_in[128:XROWS, sl])
            return xt1, xt2

        def conv123(ch, xt):
            xt1, xt2 = xt
            y3 = y3_bufs[ch % 2]

            # ---- conv1 (bias via xt2 ones row); w1a then w1b so each
            # stationary is loaded once ----
            y1 = acts.tile([128, BC], F16, tag="y1", name="y1")
            p1 = psA.tile([128, 4, 512], F32, tag="psA", name="p1")
            for t in range(2):
                nc.tensor.matmul(p1[:, t], w1a[:], xt1[:, t * 512:(t + 1) * 512],
                                 start=True, stop=False)
            for t in range(2):
                nc.tensor.matmul(p1[:, t], w1b[:], xt2[:, t * 512:(t + 1) * 512],
                                 start=False, stop=True)
            evac_prelu(_C123, y1[:].rearrange("p (a b) -> p a b", a=2),
                       p1[:, 0:2], "c1")

            # ---- conv2 (scalar evac, native bias) ----
            y2 = acts.tile([64, BC], F16, tag="y2", name="y2")
            p2 = psA.tile([64, 4, 512], F32, tag="psA", name="p2")
            for t in range(2):
                nc.tensor.matmul(p2[:, t], w2e[:], y1[:, t * 512:(t + 1) * 512],
                                 start=True, stop=True)
            nc.scalar.activation(y2[:].rearrange("p (a b) -> p a b", a=2),
                                 p2[:, 0:2], AF.Prelu, bias=b2v[:],
                                 alpha=alpha_v[0:64, :])

            # ---- conv3 (scalar evac, native bias; writes y3 rows 0:32) ----
            p3 = psA.tile([32, 4, 512], F32, tag="psA", name="p3")
            for t in range(2):
                nc.tensor.matmul(p3[:, t], w3e[:], y2[:, t * 512:(t + 1) * 512],
                                 start=True, stop=True)
            nc.scalar.activation(y3[0:32, :].rearrange("p (a b) -> p a b", a=2),
                                 p3[:, 0:2], AF.Prelu, bias=b3v[:],
                                 alpha=alpha_v[0:32, :])

        def conv4(ch):
            """4 s-pair groups (stationary loaded once per s) into the psum
            ring; each pair evacuated in one FD=2048 op."""
            y3 = y3_bufs[ch % 2]
            ye = big.tile([128, 2, S, BC], F16, tag="ye", name="ye")
            yy = ye[:, 0]
            for sp in range(4):
                p4 = psA.tile([128, 4, 512], F32, tag="psA", name=f"p4_{sp}")
                for j in range(2):
                    s = 2 * sp + j
                    for t in range(2):
                        nc.tensor.matmul(p4[:, 2 * j + t], w4s[:, s, :],
                                         y3[:, t * 512:(t + 1) * 512],
                                         start=True, stop=True)
                dst = yy[:, 2 * sp:2 * sp + 2, :].rearrange(
                    "p s (t b) -> p (s t) b", t=2)
                if sp in _SPLIT:
                    evac_prelu("dve", dst, p4[:], f"c4_{sp}")
                else:
                    evac_prelu("scalar", dst, p4[:], f"c4_{sp}")
            return ye

        def linear(ch, ye):
            yy, eep = ye[:, 0], ye[:, 1]
            for sp in range(4):
                pl = psA.tile([128, 4, 512], F32, tag="psA", name=f"pl_{sp}")
                for j in range(2):
                    s = 2 * sp + j
                    for t in range(2):
                        nc.tensor.matmul(pl[:, 2 * j + t], wlt[:],
                                         yy[:, s, t * 512:(t + 1) * 512],
                                         start=True, stop=True)
                s0, s1 = 2 * sp, 2 * sp + 1
                if (s0 in _POLY) == (s1 in _POLY):
                    # same engine for both slices: one FD=2048 op
                    dst = eep[:, s0:s1 + 1, :].rearrange(
                        "p s (t b) -> p (s t) b", t=2)
                    emit_exp(s0, dst, pl[:])
                else:
                    for j, s in ((0, s0), (1, s1)):
                        emit_exp(s, eep[:, s].rearrange("p (a b) -> p a b",
                                                        a=2),
                                 pl[:, 2 * j:2 * j + 2])

        def tail_chunk(ch, ye):
            """Full softmax tail for one chunk: numerator product, s-tree
            folds (split vector/gpsimd), reciprocal, final mul + store."""
            yy, ee = ye[:, 0], ye[:, 1]
            nc.vector.tensor_mul(yy[:], yy[:], ee[:])
            # L1
            nc.vector.tensor_add(yy[:, 0:4, :], yy[:, 0:4, :], yy[:, 4:8, :])
            nc.gpsimd.tensor_add(ee[:, 0:2, :], ee[:, 0:2, :], ee[:, 4:6, :])
            nc.vector.tensor_add(ee[:, 2:4, :], ee[:, 2:4, :], ee[:, 6:8, :])
            # L2
            nc.vector.tensor_add(yy[:, 0:2, :], yy[:, 0:2, :], yy[:, 2:4, :])
            nc.gpsimd.tensor_add(ee[:, 0:2, :], ee[:, 0:2, :], ee[:, 2:4, :])
            # L3 + normalize
            nc.gpsimd.tensor_add(yy[:, 0, :], yy[:, 0, :], yy[:, 1, :])
            dd = tail.tile([128, BC], F32, tag="dd", name="dd")
            nc.vector.tensor_add(dd[:], ee[:, 0, :], ee[:, 1, :])
            rr = tail.tile([128, BC], F32, tag="rr", name="rr")
            nc.vector.reciprocal_approx_fast(rr[:], dd[:])
            oo = tail.tile([128, BC], F16, tag="oo", name="oo")
            nc.vector.tensor_mul(oo[:], yy[:, 0, :], rr[:])
            nc.sync.dma_start(out=out_d[:, ch * BC:(ch + 1) * BC], in_=oo[:])

        # ---- pipeline ----
        repeat = int(os.environ.get("CC_REPEAT", "1"))
        for _rep in range(repeat):
            xts = {0: load(0)}
            if NCHUNK > 1:
                xts[1] = load(1)
            conv123(0, xts[0])
            yes = {}
            for i in range(NCHUNK):
                if i + 2 < NCHUNK:
                    xts[i + 2] = load(i + 2)
                if i + 1 < NCHUNK:
                    conv123(i + 1, xts.pop(i + 1))
                yes[i] = conv4(i)
                if i - 1 >= 0:
                    linear(i - 1, yes[i - 1])
                if i - 2 >= 0:
                    tail_chunk(i - 2, yes.pop(i - 2))
            linear(NCHUNK - 1, yes[NCHUNK - 1])
            if NCHUNK - 2 >= 0:
                tail_chunk(NCHUNK - 2, yes.pop(NCHUNK - 2))
            tail_chunk(NCHUNK - 1, yes.pop(NCHUNK - 1))

    nc.compile()
    return nc


def _host_weights(w1, b1, w2, b2, w3, b3, w4, b4, wl):
    # conv-as-matmul weights; rows are (cin, s_in) flattened, cols (cout,
    # s_out) flattened; zero where the kernel tap falls outside.
    def eff(wc, cin, cout):
        m = np.zeros((cin * S, cout * S), np.float32)
        for co in range(cout):
            for ci in range(cin):
                for k in range(3):
                    for so in range(S):
                        si = so + k - 1
                        if 0 <= si < S:
                            m[ci * S + si, co * S + so] = wc[co, ci, k]
        return m

    w1e = eff(w1, 29, 16)                       # [232, 128]
    w1b = np.zeros((105, 128), np.float32)
    w1b[0:104] = w1e[128:232]
    w1b[104] = np.repeat(b1, S)                 # ones-row bias
    w2e = eff(w2, 16, 8)                        # [128, 64]
    w3e = eff(w3, 8, 4)                         # [64, 32]

    # conv4 stationaries: one [33, 128] per output s; row 32 = bias.
    w4s = np.zeros((33, 8, 128), np.float32)
    for s in range(S):
        for c3 in range(4):
            for s3 in range(S):
                k = s3 - s + 1
                if 0 <= k < 3:
                    w4s[c3 * S + s3, s, :] = w4[:, c3, k]
    w4s[32, :, :] = b4[None, :]

    return dict(
        w1a=w1e[:128].astype(np.float16),
        w1b=w1b.astype(np.float16),
        w2e=w2e.astype(np.float16),
        w3e=w3e.astype(np.float16),
        w4s=np.ascontiguousarray(w4s.reshape(33, 8 * 128)).astype(np.float16),
        wlt=np.ascontiguousarray(wl.T).astype(np.float16),
        b2v=np.repeat(b2, S).reshape(64, 1).astype(np.float32),
        b3v=np.repeat(b3, S).reshape(32, 1).astype(np.float32),
    )


def _host_x(x):
    # [B, C, S] f32 -> transposed f16 [(c s)+ones, B]
    xt = np.empty((XROWS, B), np.float16)
    xt[0:CS] = np.asarray(x, np.float32).reshape(B, CS).T.astype(np.float16)
    xt[CS] = 1.0
    return xt


_NC_CACHE = None


def kernel(x, w1, b1, w2, b2, w3, b3, w4, b4, wl, bl):
    global _NC_CACHE
    xt = _host_x(x)
    wmap = _host_weights(
        np.asarray(w1, np.float32), np.asarray(b1, np.float32),
        np.asarray(w2, np.float32), np.asarray(b2, np.float32),
        np.asarray(w3, np.float32), np.asarray(b3, np.float32),
        np.asarray(w4, np.float32), np.asarray(b4, np.float32),
        np.asarray(wl, np.float32))
    # bl is constant along the softmax axis -> cancels; intentionally unused.

    if _NC_CACHE is None:
        _NC_CACHE = _build_nc()
    nc = _NC_CACHE

    core_ids = list(range(NCORES))
    in_maps = []
    for i in core_ids:
        m = {"xt": np.ascontiguousarray(xt[:, i * BPC:(i + 1) * BPC])}
        m.update(wmap)
        in_maps.append(m)
    res = run_bass_kernel_spmd(nc, in_maps, core_ids)
    outs = [res.results[i]["out"].T for i in range(NCORES)]
    return np.concatenate(outs, axis=0).astype(np.float32)
